# revision 1
# baseline (speedup 1.0000x reference)
"""GNN message-passing kernel for Trainium2, 8 NeuronCores (SPMD + collectives).

Sharding: nodes by contiguous range (6250/core, padded to 6272 = 56 windows of
112); edges by dst owner, sorted (core, src-half, window), runs padded to x128.
Per layer the message MLP is decomposed:
    m_pre = h[dst]@W1a + h[src]@W1b + ea@(We@W1c) + const
    aggr  = segsum(relu(m_pre)) ; the @mW2 is folded into the update weights.
h[dst]@W1a + ea@Weff is ONE matmul per 128-edge tile (stacked lhsT: 112 one-hot
rows + 16 eaT rows).  h[src]@W1b is gathered from an AllGathered bf16 B-table
with dma_gather (int16 idx -> lo/hi table split).  Aggregation is a matmul with
the one-hot S tile accumulating into a per-window PSUM.  Readout pooling is
masked reduction + AllReduce.
"""
import numpy as np
import ml_dtypes

import concourse.bass as bass
import concourse.bacc as bacc
import concourse.mybir as mybir
import concourse.tile as tile

BF16 = ml_dtypes.bfloat16

CFG = dict(
    N=50000, E=400000, B=8, ND=32, ED=16, H=128, L=3, NC=8,
    WSEG=96, LOCORES=5, GTILES=8, QUAD=4,
)


def _derive(cfg):
    d = dict(cfg)
    d["NPC"] = d["N"] // d["NC"]
    d["NW"] = -(-d["NPC"] // d["WSEG"])
    d["SLOT"] = d["NW"] * d["WSEG"]
    d["NSTAR"] = d["NC"] * d["SLOT"]
    d["LO"] = d["LOCORES"] * d["SLOT"]
    assert d["LO"] < 32768 and d["NSTAR"] - d["LO"] < 32768
    assert d["N"] % d["NC"] == 0
    return d


def prep_host(inputs, cfg):
    c = _derive(cfg)
    N, E, B, ND, ED, H, L, NC = (c[k] for k in "N E B ND ED H L NC".split())
    NPC, NW, WSEG, SLOT, LO = c["NPC"], c["NW"], c["WSEG"], c["SLOT"], c["LO"]

    src = np.asarray(inputs["edge_index"][0], np.int64)
    dst = np.asarray(inputs["edge_index"][1], np.int64)
    batch = np.asarray(inputs["batch"], np.int64)
    x = np.asarray(inputs["x"], np.float32)
    ea = np.asarray(inputs["edge_attr"], np.float32)

    core = dst // NPC
    dloc = dst - core * NPC
    w = dloc // WSEG
    s = dloc - w * WSEG
    srow = (src // NPC) * SLOT + (src - (src // NPC) * NPC)
    sweep = (srow >= LO).astype(np.int64)

    order = np.lexsort((w, sweep, core))
    core_o, sweep_o, w_o, s_o, srow_o = (a[order] for a in (core, sweep, w, s, srow))
    eid_o = order

    grp = (core_o * 2 + sweep_o) * NW + w_o
    counts = np.bincount(grp, minlength=NC * 2 * NW).reshape(NC, 2, NW)
    TL = np.maximum(1, -(-counts[:, 0, :].max(axis=0) // 128)).astype(int)
    TH = (-(-counts[:, 1, :].max(axis=0) // 128)).astype(int)
    TLtot, THtot = int(TL.sum()), int(TH.sum())
    TT = TLtot + THtot
    offL = np.concatenate([[0], np.cumsum(TL)[:-1]]).astype(np.int64)
    offH = np.concatenate([[0], np.cumsum(TH)[:-1]]).astype(np.int64) + TLtot

    first = np.zeros(NC * 2 * NW, np.int64)
    csum = np.cumsum(np.bincount(grp, minlength=NC * 2 * NW))
    first[1:] = csum[:-1]
    rank = np.arange(E) - first[grp]
    runbase = np.where(sweep_o == 0, offL[w_o], offH[w_o]) * 128
    pos = runbase + rank

    NP = TT * 128
    SRCI = np.zeros((NC, NP), np.int16)
    DCT = np.full((NC, NP), -1.0, np.float32)
    EAP = np.zeros((NC, NP, ED), np.float32)
    idx_lin = np.where(sweep_o == 0, srow_o, srow_o - LO)
    SRCI[core_o, pos] = idx_lin.astype(np.int16)
    DCT[core_o, pos] = s_o.astype(np.float32)
    EAP[core_o, pos] = ea[eid_o]

    def wrap(a):
        return np.ascontiguousarray(np.tile(a.reshape(-1, 16).T, (8, 1)).astype(np.int16))
    idxL = np.stack([wrap(SRCI[cc, :TLtot * 128]) for cc in range(NC)])
    idxH = (np.stack([wrap(SRCI[cc, TLtot * 128:]) for cc in range(NC)])
            if THtot > 0 else np.zeros((NC, 128, 8), np.int16))

    DCTt = np.ascontiguousarray(DCT.reshape(NC, TT, 128).transpose(0, 2, 1).astype(BF16))
    DCTrow = np.ascontiguousarray(DCT.astype(BF16)[:, None, :])
    eaT = np.ascontiguousarray(EAP.transpose(0, 2, 1).astype(BF16))

    xT = np.zeros((NC, ND, SLOT), np.float32)
    deg = np.zeros((NC, 1, SLOT), np.float32)
    gsel = np.full((NC, 1, SLOT), -1.0, np.float32)
    dcnt = np.bincount(dst, minlength=N).astype(np.float32)
    for cc in range(NC):
        xT[cc, :, :NPC] = x[cc * NPC:(cc + 1) * NPC].T
        deg[cc, 0, :NPC] = dcnt[cc * NPC:(cc + 1) * NPC]
        gsel[cc, 0, :NPC] = batch[cc * NPC:(cc + 1) * NPC].astype(np.float32)
    xT, degT, gsel = xT.astype(BF16), deg.astype(BF16), gsel.astype(BF16)

    gcnt = np.bincount(batch, minlength=B).astype(np.float32)
    invc = (1.0 / np.maximum(gcnt, 1.0)).astype(np.float32)[None, :]

    f = lambda a: np.asarray(a, np.float32)
    Wn, bn = f(inputs["Wn"]), f(inputs["bn"])
    We, be = f(inputs["We"]), f(inputs["be"])
    Wg, bg = f(inputs["Wg"]), f(inputs["bg"])
    mW1, mb1 = f(inputs["mW1"]), f(inputs["mb1"])
    mW2, mb2 = f(inputs["mW2"]), f(inputs["mb2"])
    uW1, ub1 = f(inputs["uW1"]), f(inputs["ub1"])
    uW2, ub2 = f(inputs["uW2"]), f(inputs["ub2"])
    rW1, rb1 = f(inputs["rW1"]), f(inputs["rb1"])
    rW2, rb2 = f(inputs["rW2"]), f(inputs["rb2"])
    rW3, rb3 = f(inputs["rW3"]), f(inputs["rb3"])
    gf = f(inputs["global_feature"])

    wts = {}
    wts["emb_Wn"] = Wn.astype(BF16)
    wts["bn_col"] = np.ascontiguousarray(bn[:, None])
    for l in range(L):
        W1a, W1b, W1c = mW1[l][:H], mW1[l][H:2 * H], mW1[l][2 * H:]
        wts[f"W1a_{l}"] = W1a.astype(BF16)
        wts[f"W1b_{l}"] = W1b.astype(BF16)
        wts[f"Weff_{l}"] = (We @ W1c).astype(BF16)
        wts[f"cst_{l}"] = np.ascontiguousarray((be @ W1c + mb1[l])[None, :])
        wts[f"uW1h_{l}"] = uW1[l][:H].astype(BF16)
        wts[f"uW1a_{l}"] = (mW2[l] @ uW1[l][H:]).astype(BF16)
        wts[f"vec_{l}"] = np.ascontiguousarray((mb2[l] @ uW1[l][H:])[None, :]).astype(BF16)
        wts[f"ub1_{l}"] = np.ascontiguousarray(ub1[l][:, None])
        wts[f"uW2_{l}"] = uW2[l].astype(BF16)
        wts[f"ub2_{l}"] = np.ascontiguousarray(ub2[l][:, None])
    wts["rW1p"] = rW1[:H].astype(BF16)
    wts["rW1g"] = rW1[H:].astype(BF16)
    wts["rb1_col"] = np.ascontiguousarray(rb1[:, None])
    wts["rW2"] = rW2.astype(BF16)
    wts["rb2_col"] = np.ascontiguousarray(rb2[:, None])
    wts["rW3"] = rW3.astype(BF16)
    wts["rb3_col"] = np.ascontiguousarray(rb3[:, None])
    wts["Wg_row"] = Wg.astype(BF16)
    wts["bg_col"] = np.ascontiguousarray(bg[:, None])
    wts["gfT"] = np.ascontiguousarray(gf.T).astype(BF16)
    wts["invc"] = invc
    wts["iota_r"] = np.arange(WSEG, dtype=np.float32).astype(BF16)[None, :]
    wts["iota_c"] = np.arange(128, dtype=np.float32).astype(BF16)[:, None]

    meta = dict(TL=[int(t) for t in TL], TH=[int(t) for t in TH],
                TLtot=TLtot, THtot=THtot, TT=TT, derived=c)
    percore = dict(idxL=idxL, idxH=idxH, DCTt=DCTt, DCTrow=DCTrow, eaT=eaT,
                   xT=xT, degT=degT, gsel=gsel)
    return meta, percore, wts


def _bcast_ap(dram_tensor, lo, hi, parts):
    """Manual AP: DRAM [1, n] slice replicated across `parts` partitions."""
    ap = dram_tensor[0:1, lo:hi]
    return bass.AP(ap.tensor, ap.offset, [[0, parts], [1, hi - lo]])


def build_bass(meta, wts_np, use_sliced_hi=False):
    import os as _os
    KPHASE = int(_os.environ.get("KPHASE", "4"))
    c = meta["derived"]
    B, ND, ED, H, L, NC = (c[k] for k in "B ND ED H L NC".split())
    NW, WSEG, SLOT, NSTAR, LO = c["NW"], c["WSEG"], c["SLOT"], c["NSTAR"], c["LO"]
    GT, QUAD = c["GTILES"], c["QUAD"]
    TL, TH, TLtot, THtot, TT = (meta[k] for k in ("TL", "TH", "TLtot", "THtot", "TT"))
    HI = NSTAR - LO
    MAXT = max(max(TL), max(TH) if TH else 0)
    f32, bf16, i16 = mybir.dt.float32, mybir.dt.bfloat16, mybir.dt.int16
    RELU = mybir.ActivationFunctionType.Relu
    IDENT = mybir.ActivationFunctionType.Identity
    ADD, MULT = mybir.AluOpType.add, mybir.AluOpType.mult
    EQ = mybir.AluOpType.is_equal

    nc = bacc.Bacc("TRN2", target_bir_lowering=False, debug=False, num_devices=NC)

    NP = TT * 128
    t_idxL = nc.dram_tensor("idxL", [128, max(TLtot, 1) * 8], i16, kind="ExternalInput")
    t_idxH = nc.dram_tensor("idxH", [128, max(THtot, 1) * 8], i16, kind="ExternalInput")
    t_dctt = nc.dram_tensor("DCTt", [128, TT], bf16, kind="ExternalInput")
    t_dctrow = nc.dram_tensor("DCTrow", [1, NP], bf16, kind="ExternalInput")
    t_eaT = nc.dram_tensor("eaT", [ED, NP], bf16, kind="ExternalInput")
    t_xT = nc.dram_tensor("xT", [ND, SLOT], bf16, kind="ExternalInput")
    t_degT = nc.dram_tensor("degT", [1, SLOT], bf16, kind="ExternalInput")
    t_gsel = nc.dram_tensor("gsel", [1, SLOT], bf16, kind="ExternalInput")
    wt = {k: nc.dram_tensor(k, list(v.shape),
                            bf16 if v.dtype == BF16 else f32, kind="ExternalInput")
          for k, v in wts_np.items()}
    t_out = nc.dram_tensor("out", [1, B], f32, kind="ExternalOutput")

    offL = [int(v) for v in np.concatenate([[0], np.cumsum(TL)[:-1]])]
    offH = [int(v) for v in (np.concatenate([[0], np.cumsum(TH)[:-1]]) + TLtot)]

    with tile.TileContext(nc) as tc:
        with tc.tile_pool(name="const", bufs=1) as cpool, \
             tc.tile_pool(name="data", bufs=1) as dpool, \
             tc.tile_pool(name="comb", bufs=2) as combp, \
             tc.tile_pool(name="sw", bufs=2) as swp, \
             tc.tile_pool(name="bg", bufs=3) as bgp, \
             tc.tile_pool(name="wk", bufs=3) as wkp, \
             tc.tile_pool(name="stp", bufs=3) as stp, \
             tc.tile_pool(name="ps", bufs=2, space="PSUM") as psp, \
             tc.tile_pool(name="dram", bufs=2, space="DRAM") as drp, \
             tc.tile_pool(name="dram1", bufs=1, space="DRAM") as drp1:

            w_sb = {}
            for k, v in wts_np.items():
                tl = cpool.tile(list(v.shape), bf16 if v.dtype == BF16 else f32, tag=k)
                nc.sync.dma_start(out=tl[:], in_=wt[k][:])
                w_sb[k] = tl
            iota_r = cpool.tile([128, WSEG], bf16, tag="iota_r_rep")
            nc.sync.dma_start(out=iota_r[:], in_=_bcast_ap(wt["iota_r"], 0, WSEG, 128))
            invc_rep = cpool.tile([128, B], f32, tag="invc_rep")
            nc.sync.dma_start(out=invc_rep[:], in_=_bcast_ap(wt["invc"], 0, B, 128))
            cst_rep = {}
            for l in range(L):
                t = cpool.tile([128, H], f32, tag=f"cst_rep{l}")
                nc.sync.dma_start(out=t[:], in_=_bcast_ap(wt[f"cst_{l}"], 0, H, 128))
                cst_rep[l] = t

            hT = dpool.tile([128, SLOT], f32, tag="hT")
            hTb = dpool.tile([128, SLOT], bf16, tag="hTb")
            aggr = dpool.tile([128, SLOT], bf16, tag="aggr")
            AW = dpool.tile([128, NW * 128], bf16, tag="AW")
            dctt = dpool.tile([128, TT], bf16, tag="dctt")
            nc.sync.dma_start(out=dctt[:], in_=t_dctt[:])
            xT_sb = dpool.tile([ND, SLOT], bf16, tag="xT")
            nc.sync.dma_start(out=xT_sb[:], in_=t_xT[:])
            degT_sb = dpool.tile([1, SLOT], bf16, tag="degT")
            nc.sync.dma_start(out=degT_sb[:], in_=t_degT[:])
            idxL_sb = dpool.tile([128, max(TLtot, 1) * 8], i16, tag="idxL")
            nc.sync.dma_start(out=idxL_sb[:], in_=t_idxL[:])
            idxH_sb = dpool.tile([128, max(THtot, 1) * 8], i16, tag="idxH")
            if THtot > 0:
                nc.sync.dma_start(out=idxH_sb[:], in_=t_idxH[:])

            def strips512():
                o = 0
                while o < SLOT:
                    fz = min(512, SLOT - o)
                    yield o, fz
                    o += fz

            # embedding
            for o, fz in strips512():
                ps = psp.tile([128, 512], f32, space="PSUM", tag="pu")
                nc.tensor.matmul(out=ps[:, :fz], lhsT=w_sb["emb_Wn"][:],
                                 rhs=xT_sb[:, o:o + fz], start=True, stop=True)
                nc.scalar.activation(out=hT[:, o:o + fz], in_=ps[:, :fz],
                                     func=IDENT, bias=w_sb["bn_col"][:])
                nc.vector.tensor_copy(out=hTb[:, o:o + fz], in_=hT[:, o:o + fz])

            for l in range(L if KPHASE >= 2 else 0):
                # B table + AllGather
                b_own = drp.tile([SLOT, H], bf16, tag="b_own")
                b_full = drp.tile([NSTAR, H], bf16, tag="b_full")
                for so in range(0, SLOT, 128):
                    fz = min(128, SLOT - so)
                    ps = psp.tile([128, 128], f32, space="PSUM", tag="p128")
                    nc.tensor.matmul(out=ps[0:fz, :], lhsT=hTb[:, so:so + fz],
                                     rhs=w_sb[f"W1b_{l}"][:], start=True, stop=True)
                    bstr = stp.tile([128, H], bf16, tag="bstr")
                    nc.vector.tensor_tensor(out=bstr[0:fz, :], in0=ps[0:fz, :],
                                            in1=cst_rep[l][0:fz, :], op=ADD)
                    nc.sync.dma_start(out=b_own[so:so + fz, :], in_=bstr[0:fz, :])
                nc.gpsimd.collective_compute(
                    "AllGather", mybir.AluOpType.bypass,
                    replica_groups=[list(range(NC))],
                    ins=[b_own.opt()], outs=[b_full.opt()])
                if use_sliced_hi:
                    b_lo_ap, b_hi_ap = b_full[0:LO, :], b_full[LO:NSTAR, :]
                else:
                    b_hi = drp.tile([HI, H], bf16, tag="b_hi")
                    nc.sync.dma_start(out=b_hi[:], in_=b_full[LO:NSTAR, :])
                    b_lo_ap, b_hi_ap = b_full[0:LO, :], b_hi[:]

                # A windows + Weff rows
                for w in range(NW if KPHASE >= 3 else 0):
                    ps = psp.tile([128, 128], f32, space="PSUM", tag="p128")
                    nc.tensor.matmul(out=ps[0:WSEG, :],
                                     lhsT=hTb[:, w * WSEG:(w + 1) * WSEG],
                                     rhs=w_sb[f"W1a_{l}"][:], start=True, stop=True)
                    nc.scalar.activation(out=AW[0:WSEG, w * 128:w * 128 + 128],
                                         in_=ps[0:WSEG, :], func=IDENT)
                    nc.vector.tensor_copy(out=AW[WSEG:WSEG + ED, w * 128:w * 128 + 128],
                                          in_=w_sb[f"Weff_{l}"][:])

                def sweep(sweep_id, Tarr, offarr, tstart, idx_sb, tab_ap):
                    Ttot_s = int(sum(Tarr))
                    if Ttot_s == 0:
                        return
                    w_of = []
                    for w in range(NW):
                        w_of += [w] * Tarr[w]
                    bg_tiles = {}
                    for g0 in range(0, Ttot_s, GT):
                        gn = min(GT, Ttot_s - g0)
                        bgt = bgp.tile([128, GT, 128], bf16, tag="bgt")
                        nc.gpsimd.dma_gather(
                            out_ap=bgt[:, 0:gn, :], in_ap=tab_ap,
                            idxs_ap=idx_sb[:, g0 * 8:(g0 + gn) * 8],
                            num_idxs=gn * 128, num_idxs_reg=gn * 128,
                            elem_size=H, single_packet=False)
                        bg_tiles[g0] = bgt
                    comb_t, s_t = {}, {}
                    for w in range(NW):
                        Tw = Tarr[w]
                        if Tw == 0:
                            continue
                        np0 = (offarr[w]) * 128
                        cmb = combp.tile([128, MAXT * 128], bf16, tag="cmb")
                        drt = wkp.tile([WSEG, MAXT * 128], bf16, tag="drt")
                        nc.sync.dma_start(
                            out=drt[:, 0:Tw * 128],
                            in_=_bcast_ap(t_dctrow, np0, np0 + Tw * 128, WSEG))
                        nc.vector.tensor_tensor(
                            out=cmb[0:WSEG, 0:Tw * 128],
                            in0=w_sb["iota_c"][0:WSEG, :].to_broadcast([WSEG, Tw * 128]),
                            in1=drt[:, 0:Tw * 128], op=EQ)
                        nc.sync.dma_start(out=cmb[WSEG:WSEG + ED, 0:Tw * 128],
                                          in_=t_eaT[:, np0:np0 + Tw * 128])
                        st = swp.tile([128, MAXT, WSEG], bf16, tag="st")
                        nc.vector.tensor_tensor(
                            out=st[:, 0:Tw, :],
                            in0=dctt[:, offarr[w]:offarr[w] + Tw].to_broadcast([128, Tw, WSEG]),
                            in1=iota_r[:].rearrange("p (a w) -> p a w", a=1).to_broadcast([128, Tw, WSEG]),
                            op=EQ)
                        comb_t[w], s_t[w] = cmb, st
                    t = 0
                    pag_of = {}
                    while t < Ttot_s:
                        qn = min(QUAD, Ttot_s - t)
                        psq = psp.tile([128, QUAD * 128], f32, space="PSUM", tag="pq")
                        for j in range(qn):
                            w = w_of[t + j]
                            ltw = (t + j) - (offarr[w] - tstart)
                            nc.tensor.matmul(
                                out=psq[:, j * 128:(j + 1) * 128],
                                lhsT=comb_t[w][0:WSEG + ED, ltw * 128:(ltw + 1) * 128],
                                rhs=AW[0:WSEG + ED, w * 128:(w + 1) * 128],
                                start=True, stop=True)
                        g0 = (t // GT) * GT
                        bgt = bg_tiles[g0]
                        ms = wkp.tile([128, QUAD * 128], bf16, tag="ms")
                        nc.vector.tensor_tensor(
                            out=ms[:, 0:qn * 128], in0=psq[:, 0:qn * 128],
                            in1=bgt[:, t - g0:t - g0 + qn, :].rearrange("p a b -> p (a b)"),
                            op=ADD)
                        rs = wkp.tile([128, QUAD * 128], bf16, tag="rs")
                        nc.scalar.activation(out=rs[:, 0:qn * 128], in_=ms[:, 0:qn * 128],
                                             func=RELU)
                        for j in range(qn):
                            w = w_of[t + j]
                            ltw = (t + j) - (offarr[w] - tstart)
                            pagt = pag_of.get(w)
                            if pagt is None:
                                pagt = psp.tile([128, WSEG], f32, space="PSUM", tag="pagt")
                                pag_of[w] = pagt
                            nc.tensor.matmul(
                                out=pagt[:], lhsT=rs[:, j * 128:(j + 1) * 128],
                                rhs=s_t[w][:, ltw, :],
                                start=(ltw == 0), stop=(ltw == Tarr[w] - 1))
                            if ltw == Tarr[w] - 1:
                                if sweep_id == 0:
                                    nc.scalar.activation(
                                        out=aggr[:, w * WSEG:(w + 1) * WSEG],
                                        in_=pagt[:], func=IDENT)
                                else:
                                    nc.vector.tensor_tensor(
                                        out=aggr[:, w * WSEG:(w + 1) * WSEG],
                                        in0=aggr[:, w * WSEG:(w + 1) * WSEG],
                                        in1=pagt[:], op=ADD)
                                del pag_of[w]
                        t += qn

                if KPHASE >= 3:
                    sweep(0, TL, offL, 0, idxL_sb, b_lo_ap)
                    sweep(1, TH, offH, TLtot, idxH_sb, b_hi_ap)

                # update MLP (mW2 folded into uW1a; deg term via K=1 matmul)
                for o, fz in (strips512() if KPHASE >= 4 else []):
                    ps = psp.tile([128, 512], f32, space="PSUM", tag="pu")
                    nc.tensor.matmul(out=ps[:, :fz], lhsT=w_sb[f"uW1h_{l}"][:],
                                     rhs=hTb[:, o:o + fz], start=True, stop=False)
                    nc.tensor.matmul(out=ps[:, :fz], lhsT=w_sb[f"uW1a_{l}"][:],
                                     rhs=aggr[:, o:o + fz], start=False, stop=False)
                    nc.tensor.matmul(out=ps[:, :fz], lhsT=w_sb[f"vec_{l}"][:],
                                     rhs=degT_sb[:, o:o + fz], start=False, stop=True)
                    t1 = stp.tile([128, 512], bf16, tag="t1")
                    nc.scalar.activation(out=t1[:, :fz], in_=ps[:, :fz], func=RELU,
                                         bias=w_sb[f"ub1_{l}"][:])
                    ps2 = psp.tile([128, 512], f32, space="PSUM", tag="pu")
                    nc.tensor.matmul(out=ps2[:, :fz], lhsT=w_sb[f"uW2_{l}"][:],
                                     rhs=t1[:, :fz], start=True, stop=True)
                    nc.vector.scalar_tensor_tensor(
                        out=hT[:, o:o + fz], in0=ps2[:, :fz],
                        scalar=w_sb[f"ub2_{l}"][:], in1=hT[:, o:o + fz],
                        op0=ADD, op1=ADD)
                    if l < L - 1:
                        nc.vector.tensor_copy(out=hTb[:, o:o + fz], in_=hT[:, o:o + fz])

            # pooled readout (masked sums, SPMD-uniform)
            gsel_rep = dpool.tile([128, SLOT], bf16, tag="gsel_rep")
            nc.sync.dma_start(out=gsel_rep[:], in_=_bcast_ap(t_gsel, 0, SLOT, 128))
            mask_sb = dpool.tile([128, SLOT], bf16, tag="mask_sb")
            prod_sb = dpool.tile([128, SLOT], f32, tag="prod_sb")
            pool_pt = dpool.tile([128, B], f32, tag="pool_pt")
            nc.vector.memset(pool_pt[:], 0.0)
            for g in range(B):
                nc.vector.tensor_scalar(out=mask_sb[:], in0=gsel_rep[:],
                                        scalar1=float(g), scalar2=None, op0=EQ)
                nc.vector.tensor_tensor(out=prod_sb[:], in0=mask_sb[:],
                                        in1=hT[:], op=MULT)
                nc.vector.reduce_sum(out=pool_pt[:, g:g + 1], in_=prod_sb[:],
                                     axis=mybir.AxisListType.X)
            ari = drp1.tile([128, B], f32, tag="ari")
            aro = drp1.tile([128, B], f32, tag="aro")
            nc.gpsimd.dma_start(out=ari[:], in_=pool_pt[:])
            nc.gpsimd.collective_compute(
                "AllReduce", ADD, replica_groups=[list(range(NC))],
                ins=[ari.opt()], outs=[aro.opt()])
            poolsum = dpool.tile([128, B], f32, tag="poolsum")
            nc.gpsimd.dma_start(out=poolsum[:], in_=aro[:])
            pooled = dpool.tile([128, B], bf16, tag="pooled")
            nc.vector.tensor_tensor(out=pooled[:], in0=poolsum[:], in1=invc_rep[:],
                                    op=MULT)
            psg = psp.tile([128, B], f32, space="PSUM", tag="pu")
            nc.tensor.matmul(out=psg[:], lhsT=w_sb["Wg_row"][:], rhs=w_sb["gfT"][:],
                             start=True, stop=True)
            g_sb = dpool.tile([128, B], bf16, tag="g_sb")
            nc.scalar.activation(out=g_sb[:], in_=psg[:], func=IDENT,
                                 bias=w_sb["bg_col"][:])
            ps1 = psp.tile([128, B], f32, space="PSUM", tag="pu")
            nc.tensor.matmul(out=ps1[:], lhsT=w_sb["rW1p"][:], rhs=pooled[:],
                             start=True, stop=False)
            nc.tensor.matmul(out=ps1[:], lhsT=w_sb["rW1g"][:], rhs=g_sb[:],
                             start=False, stop=True)
            t1r = dpool.tile([128, B], bf16, tag="t1r")
            nc.scalar.activation(out=t1r[:], in_=ps1[:], func=RELU,
                                 bias=w_sb["rb1_col"][:])
            ps2r = psp.tile([64, B], f32, space="PSUM", tag="pu")
            nc.tensor.matmul(out=ps2r[:], lhsT=w_sb["rW2"][:], rhs=t1r[:],
                             start=True, stop=True)
            t2r = dpool.tile([64, B], bf16, tag="t2r")
            nc.scalar.activation(out=t2r[:], in_=ps2r[:], func=RELU,
                                 bias=w_sb["rb2_col"][:])
            ps3 = psp.tile([1, B], f32, space="PSUM", tag="pu")
            nc.tensor.matmul(out=ps3[:], lhsT=w_sb["rW3"][:], rhs=t2r[:],
                             start=True, stop=True)
            out_sb = dpool.tile([1, B], f32, tag="out_sb")
            nc.scalar.activation(out=out_sb[:], in_=ps3[:], func=IDENT,
                                 bias=w_sb["rb3_col"][:])
            nc.sync.dma_start(out=t_out[:], in_=out_sb[:])

    nc.compile()
    return nc


def make_in_maps(meta, percore, wts, cfg):
    NC = cfg["NC"]
    in_maps = []
    for c in range(NC):
        m = {k: np.ascontiguousarray(v) for k, v in wts.items()}
        m.update(idxL=percore["idxL"][c], idxH=percore["idxH"][c],
                 DCTt=percore["DCTt"][c], DCTrow=percore["DCTrow"][c],
                 eaT=percore["eaT"][c], xT=percore["xT"][c],
                 degT=percore["degT"][c], gsel=percore["gsel"][c])
        in_maps.append(m)
    return in_maps


def run(inputs, cfg=None, trace=False, tmpdir=None):
    cfg = cfg or CFG
    meta, percore, wts = prep_host(inputs, cfg)
    nc = build_bass(meta, wts)
    in_maps = make_in_maps(meta, percore, wts, cfg)
    from concourse.bass_utils import run_bass_kernel_spmd
    kw = {}
    if tmpdir:
        kw["tmpdir"] = tmpdir
    res = run_bass_kernel_spmd(nc, in_maps, core_ids=list(range(cfg["NC"])),
                               trace=trace, **kw)
    out = np.asarray(res.results[0]["out"], np.float32).reshape(cfg["B"], 1)
    return out, res


def kernel(**inputs) -> np.ndarray:
    out, _ = run(inputs)
    return out

